# revision 14
# baseline (speedup 1.0000x reference)
"""DiffPool layer (GCN embed + GCN assign + softmax + S^T x / S^T A S) on 8 trn2 cores.

Sharding: nodes split into 8 contiguous ranges of 2048 (one per core).
Pass 1 (launch 1): edges sharded by dst owner; per 128-node block, messages
x[src]*w are gathered edge-major via dma_gather and segment-summed into the
block via weighted-one-hot matmuls accumulating in PSUM (out = gathered^T @ onehot
-> x_agg^T feature-major). Self-loops enter the same PSUM group as a weighted
diagonal matmul. Then logits/emb matmuls + softmax per block, and the
x_pooled = S^T x_emb partial accumulates across blocks in PSUM.
Pass 2 (launch 2): unique edges sharded by src owner; Z = A S built per block
from gathered S[dst] with unweighted one-hot matmuls (lhsT=onehot), then
A_pooled partial = S_own^T Z accumulates in PSUM. Host sums the 8 partials
and concatenates the S shards.
"""

import sys

sys.path.insert(0, "/opt/trn_rl_repo")

import numpy as np

import concourse.bacc as bacc
import concourse.mybir as mybir
from concourse.tile import TileContext
from concourse.bass_utils import run_bass_kernel_spmd

N_NODES = 16384
N_EDGES = 524288
F = 128  # in/hid/cluster channels all 128
NCORES = 8
NPC = N_NODES // NCORES  # nodes per core
BPC = NPC // 128  # 128-node blocks per core

AT = mybir.AluOpType
ACTF = mybir.ActivationFunctionType
F32 = mybir.dt.float32
I16 = mybir.dt.int16

_LAST_SIM_NS = {}


def _wrap16(a):
    """[n*16] int16 -> [128, n] gather-index layout (16-partition wrap, x8)."""
    t = a.reshape(-1, 16).T
    return np.ascontiguousarray(np.tile(t, (8, 1)))


def _wrap128(a):
    """[n*128] -> [128, n] edge-major (gather output position order)."""
    return np.ascontiguousarray(a.reshape(-1, 128).T)


def _group_edges(tgt, aux_arrays, n_aux_pad):
    """Group edges by target-node block, pad each (core, block) list to the
    shared per-block-position chunk count.

    tgt: [E] target node of each edge (the segment-sum key, already the
         sharding key). aux_arrays: list of per-edge arrays to carry along.
    n_aux_pad: pad value per aux array.
    Returns (cmax [BPC], per-core dict of arrays).
    """
    order = np.argsort(tgt, kind="stable")
    tgt_s = tgt[order]
    aux_s = [a[order] for a in aux_arrays]
    gb = tgt_s >> 7  # global block 0..127
    counts = np.bincount(gb, minlength=NCORES * BPC)
    starts = np.concatenate([[0], np.cumsum(counts)])
    cmax = np.zeros(BPC, dtype=np.int64)
    for c in range(NCORES):
        for b in range(BPC):
            cnt = counts[c * BPC + b]
            cmax[b] = max(cmax[b], -(-cnt // 128))
    cores = []
    for c in range(NCORES):
        loc_list = []  # local one-hot target (or -1 pad)
        aux_lists = [[] for _ in aux_arrays]
        for b in range(BPC):
            g = c * BPC + b
            s, e = starts[g], starts[g + 1]
            cnt = e - s
            pad = cmax[b] * 128 - cnt
            loc = (tgt_s[s:e] & 127).astype(np.float32)
            loc_list.append(np.concatenate([loc, np.full(pad, -1.0, np.float32)]))
            for ai, a in enumerate(aux_s):
                aux_lists[ai].append(
                    np.concatenate([a[s:e], np.full(pad, n_aux_pad[ai], a.dtype)])
                )
        cores.append(
            (np.concatenate(loc_list), [np.concatenate(al) for al in aux_lists])
        )
    return cmax, cores


def _build_pass1(cmax, zero_bias=False):
    CH1 = int(cmax.sum())
    nc = bacc.Bacc("TRN2", target_bir_lowering=False)
    xd = nc.dram_tensor("x", [N_NODES, F], F32, kind="ExternalInput")
    xod = nc.dram_tensor("xown", [NPC, F], F32, kind="ExternalInput")
    idxd = nc.dram_tensor("idx1", [128, CH1 * 8], I16, kind="ExternalInput")
    dlocd = nc.dram_tensor("dloc1", [128, CH1], F32, kind="ExternalInput")
    dsgd = nc.dram_tensor("degs1", [128, CH1], F32, kind="ExternalInput")
    dgdd = nc.dram_tensor("degd1", [128, CH1], F32, kind="ExternalInput")
    degod = nc.dram_tensor("degown", [128, BPC], F32, kind="ExternalInput")
    iotad = nc.dram_tensor("iota", [128, 128], F32, kind="ExternalInput")
    iotacd = nc.dram_tensor("iotac", [128, 1], F32, kind="ExternalInput")
    wembd = nc.dram_tensor("wemb", [F, F], F32, kind="ExternalInput")
    wasnd = nc.dram_tensor("wasn", [F, F], F32, kind="ExternalInput")
    bembd = nc.dram_tensor("bembb", [128, F], F32, kind="ExternalInput")
    basnd = nc.dram_tensor("basnb", [128, F], F32, kind="ExternalInput")
    sod = nc.dram_tensor("s_out", [NPC, F], F32, kind="ExternalOutput")
    xpd = nc.dram_tensor("xp_part", [128, 128], F32, kind="ExternalOutput")

    with TileContext(nc) as tc:
        with (
            tc.tile_pool(name="const", bufs=1) as constp,
            tc.tile_pool(name="meta", bufs=1) as metap,
            tc.tile_pool(name="gath", bufs=4) as gathp,
            tc.tile_pool(name="oh", bufs=4) as ohp,
            tc.tile_pool(name="work", bufs=4) as workp,
            tc.tile_pool(name="big", bufs=1) as bigp,
            tc.tile_pool(name="stat", bufs=4) as statp,
            tc.tile_pool(name="pagg", bufs=2, space="PSUM") as paggp,
            tc.tile_pool(name="pmm", bufs=2, space="PSUM") as pmmp,
            tc.tile_pool(name="pxp", bufs=1, space="PSUM") as pxpp,
        ):
            iota_t = constp.tile([128, 128], F32, tag="iota")
            nc.sync.dma_start(iota_t[:, :], iotad[:, :])
            iotac_t = constp.tile([128, 1], F32, tag="iotac")
            nc.sync.dma_start(iotac_t[:, :], iotacd[:, :])
            wemb_t = constp.tile([F, F], F32, tag="wemb")
            nc.sync.dma_start(wemb_t[:, :], wembd[:, :])
            wasn_t = constp.tile([F, F], F32, tag="wasn")
            nc.sync.dma_start(wasn_t[:, :], wasnd[:, :])
            bemb_t = constp.tile([128, F], F32, tag="bemb")
            nc.sync.dma_start(bemb_t[:, :], bembd[:, :])
            basn_t = constp.tile([128, F], F32, tag="basn")
            nc.sync.dma_start(basn_t[:, :], basnd[:, :])

            idx_t = metap.tile([128, CH1 * 8], I16, tag="idx")
            nc.sync.dma_start(idx_t[:, :], idxd[:, :])
            dloc_t = metap.tile([128, CH1], F32, tag="dloc")
            nc.sync.dma_start(dloc_t[:, :], dlocd[:, :])
            dsg_t = metap.tile([128, CH1], F32, tag="dsg")
            nc.sync.dma_start(dsg_t[:, :], dsgd[:, :])
            dgd_t = metap.tile([128, CH1], F32, tag="dgd")
            nc.sync.dma_start(dgd_t[:, :], dgdd[:, :])
            dego_t = metap.tile([128, BPC], F32, tag="dego")
            nc.sync.dma_start(dego_t[:, :], degod[:, :])

            # edge weights w = sqrt(1/(deg[src]*deg[dst]))
            dprod_t = metap.tile([128, CH1], F32, tag="dprod")
            nc.vector.tensor_tensor(dprod_t[:, :], dsg_t[:, :], dgd_t[:, :], AT.mult)
            drec_t = metap.tile([128, CH1], F32, tag="drec")
            nc.vector.reciprocal(drec_t[:, :], dprod_t[:, :])
            w1_t = metap.tile([128, CH1], F32, tag="w1")
            nc.scalar.activation(w1_t[:, :], drec_t[:, :], ACTF.Sqrt)
            # self-loop weight 1/deg
            invdeg_t = metap.tile([128, BPC], F32, tag="invdeg")
            nc.vector.reciprocal(invdeg_t[:, :], dego_t[:, :])

            xaggT = bigp.tile([128, NPC], F32, tag="xaggT")
            S_sb = bigp.tile([128, NPC], F32, tag="S_sb")
            xemb_sb = bigp.tile([128, NPC], F32, tag="xemb_sb")

            # gather calls capped at 1024 idxs (8 chunks) — HW limit; calls
            # are decoupled from blocks and cover the global chunk stream.
            CH1 = int(cmax.sum())
            CPG = 8  # chunks per gather call
            gtiles = {}

            def chunk_ap(ci):
                k = ci // CPG
                if k not in gtiles:
                    w = min(CPG, CH1 - k * CPG)
                    t = gathp.tile([128, w, F], F32, tag="g")
                    nc.gpsimd.dma_gather(
                        t[:, :, :],
                        xd[:, :],
                        idx_t[:, k * CPG * 8 : k * CPG * 8 + w * 8],
                        num_idxs=w * 128,
                        num_idxs_reg=w * 128,
                        elem_size=F,
                    )
                    gtiles[k] = t
                return gtiles[k][:, ci % CPG, :]

            # ---- aggregation: x_agg^T[f, v] blocks in PSUM ----
            off = 0
            for b in range(BPC):
                nch = int(cmax[b])
                bs = slice(b * 128, (b + 1) * 128)
                pa = paggp.tile([128, 128], F32, tag="pa")
                for ci in range(nch):
                    oh = ohp.tile([128, 128], F32, tag="oh")
                    col = slice(off + ci, off + ci + 1)
                    nc.vector.tensor_scalar(
                        oh[:, :], iota_t[:, :], dloc_t[:, col], w1_t[:, col],
                        AT.is_equal, AT.mult,
                    )
                    nc.tensor.matmul(
                        pa[:, :], chunk_ap(off + ci), oh[:, :],
                        start=(ci == 0), stop=False, skip_group_check=True,
                    )
                # self loop: lhsT = x_own block [v(e), f], rhs = diag(1/deg)
                dmat = ohp.tile([128, 128], F32, tag="oh")
                nc.vector.tensor_scalar(
                    dmat[:, :], iota_t[:, :], iotac_t[:, :], invdeg_t[:, b : b + 1],
                    AT.is_equal, AT.mult,
                )
                xob = workp.tile([128, F], F32, tag="xob")
                nc.sync.dma_start(xob[:, :], xod[b * 128 : (b + 1) * 128, :])
                nc.tensor.matmul(
                    pa[:, :], xob[:, :], dmat[:, :],
                    start=False, stop=True, skip_group_check=True,
                )
                nc.vector.tensor_copy(xaggT[:, bs], pa[:, :])
                off += nch

            # ---- per-block: logits -> softmax -> S ; emb -> relu ; xp accum ----
            pxp = pxpp.tile([128, 128], F32, tag="pxp")
            for b in range(BPC):
                bs = slice(b * 128, (b + 1) * 128)
                xat = xaggT[:, bs]
                pl = pmmp.tile([128, 128], F32, tag="pl")
                nc.tensor.matmul(pl[:, :], xat, wasn_t[:, :], start=True, stop=True,
                                 skip_group_check=True)
                lg = workp.tile([128, 128], F32, tag="lg")
                nc.vector.tensor_tensor(lg[:, :], pl[:, :], basn_t[:, :], AT.add)
                negm = statp.tile([128, 1], F32, tag="negm")
                nc.vector.tensor_reduce(
                    negm[:, :], lg[:, :], axis=mybir.AxisListType.X, op=AT.max,
                    negate=True,
                )
                ex = workp.tile([128, 128], F32, tag="ex")
                ssum = statp.tile([128, 1], F32, tag="ssum")
                nc.scalar.activation(
                    ex[:, :], lg[:, :], ACTF.Exp, bias=negm[:, :], accum_out=ssum[:, :]
                )
                rs = statp.tile([128, 1], F32, tag="rs")
                nc.vector.reciprocal(rs[:, :], ssum[:, :])
                nc.vector.tensor_scalar_mul(S_sb[:, bs], ex[:, :], rs[:, :])
                nc.sync.dma_start(sod[b * 128 : (b + 1) * 128, :], S_sb[:, bs])

                pe_ = pmmp.tile([128, 128], F32, tag="pe")
                nc.tensor.matmul(pe_[:, :], xat, wemb_t[:, :], start=True, stop=True,
                                 skip_group_check=True)
                t2 = workp.tile([128, 128], F32, tag="t2")
                nc.vector.tensor_tensor(t2[:, :], pe_[:, :], bemb_t[:, :], AT.add)
                nc.vector.tensor_scalar_max(xemb_sb[:, bs], t2[:, :], 0.0)
                nc.tensor.matmul(
                    pxp[:, :], S_sb[:, bs], xemb_sb[:, bs],
                    start=(b == 0), stop=(b == BPC - 1), skip_group_check=True,
                )
            xps = workp.tile([128, 128], F32, tag="xps")
            nc.vector.tensor_copy(xps[:, :], pxp[:, :])
            nc.sync.dma_start(xpd[:, :], xps[:, :])

    nc.compile()
    return nc


def _build_pass2(cmax2):
    CH2 = int(cmax2.sum())
    nc = bacc.Bacc("TRN2", target_bir_lowering=False)
    sfd = nc.dram_tensor("sfull", [N_NODES, F], F32, kind="ExternalInput")
    sod = nc.dram_tensor("sown", [NPC, F], F32, kind="ExternalInput")
    idxd = nc.dram_tensor("idx2", [128, CH2 * 8], I16, kind="ExternalInput")
    slocd = nc.dram_tensor("sloc2", [128, CH2], F32, kind="ExternalInput")
    iotad = nc.dram_tensor("iota", [128, 128], F32, kind="ExternalInput")
    apd = nc.dram_tensor("ap_part", [128, 128], F32, kind="ExternalOutput")

    with TileContext(nc) as tc:
        with (
            tc.tile_pool(name="const", bufs=1) as constp,
            tc.tile_pool(name="meta", bufs=1) as metap,
            tc.tile_pool(name="gath", bufs=4) as gathp,
            tc.tile_pool(name="oh", bufs=4) as ohp,
            tc.tile_pool(name="work", bufs=4) as workp,
            tc.tile_pool(name="pz", bufs=2, space="PSUM") as pzp,
            tc.tile_pool(name="pap", bufs=1, space="PSUM") as papp,
        ):
            iota_t = constp.tile([128, 128], F32, tag="iota")
            nc.sync.dma_start(iota_t[:, :], iotad[:, :])
            idx_t = metap.tile([128, CH2 * 8], I16, tag="idx")
            nc.sync.dma_start(idx_t[:, :], idxd[:, :])
            sloc_t = metap.tile([128, CH2], F32, tag="sloc")
            nc.sync.dma_start(sloc_t[:, :], slocd[:, :])

            CH2 = int(cmax2.sum())
            CPG = 8
            gtiles = {}

            def chunk_ap(ci):
                k = ci // CPG
                if k not in gtiles:
                    w = min(CPG, CH2 - k * CPG)
                    t = gathp.tile([128, w, F], F32, tag="g")
                    nc.gpsimd.dma_gather(
                        t[:, :, :],
                        sfd[:, :],
                        idx_t[:, k * CPG * 8 : k * CPG * 8 + w * 8],
                        num_idxs=w * 128,
                        num_idxs_reg=w * 128,
                        elem_size=F,
                    )
                    gtiles[k] = t
                return gtiles[k][:, ci % CPG, :]

            pap = papp.tile([128, 128], F32, tag="pap")
            off = 0
            for b in range(BPC):
                nch = int(cmax2[b])
                pz = pzp.tile([128, 128], F32, tag="pz")
                for ci in range(nch):
                    oh = ohp.tile([128, 128], F32, tag="oh")
                    col = slice(off + ci, off + ci + 1)
                    nc.vector.tensor_scalar(
                        oh[:, :], iota_t[:, :], sloc_t[:, col], None, AT.is_equal
                    )
                    # lhsT = onehot [e, v], rhs = gathered S[dst] [e, l] -> Z[v, l]
                    nc.tensor.matmul(
                        pz[:, :], oh[:, :], chunk_ap(off + ci),
                        start=(ci == 0), stop=(ci == nch - 1), skip_group_check=True,
                    )
                zb = workp.tile([128, 128], F32, tag="zb")
                nc.vector.tensor_copy(zb[:, :], pz[:, :])
                sob = workp.tile([128, F], F32, tag="sob")
                nc.sync.dma_start(sob[:, :], sod[b * 128 : (b + 1) * 128, :])
                nc.tensor.matmul(
                    pap[:, :], sob[:, :], zb[:, :],
                    start=(b == 0), stop=(b == BPC - 1), skip_group_check=True,
                )
                off += nch
            aps = workp.tile([128, 128], F32, tag="aps")
            nc.vector.tensor_copy(aps[:, :], pap[:, :])
            nc.sync.dma_start(apd[:, :], aps[:, :])

    nc.compile()
    return nc


def prep_pass1(x, edge_index, W_emb, b_emb, W_asn, b_asn):
    x = np.ascontiguousarray(np.asarray(x, dtype=np.float32))
    ei = np.asarray(edge_index)
    src = ei[0].astype(np.int64)
    dst = ei[1].astype(np.int64)
    W_emb = np.ascontiguousarray(np.asarray(W_emb, np.float32))
    W_asn = np.ascontiguousarray(np.asarray(W_asn, np.float32))
    b_emb = np.asarray(b_emb, np.float32)
    b_asn = np.asarray(b_asn, np.float32)

    deg = (np.bincount(dst, minlength=N_NODES) + 1).astype(np.float32)

    # ---------- pass 1 host prep: edges grouped by dst ----------
    cmax1, cores1 = _group_edges(
        dst,
        [src.astype(np.int16), deg[src], deg[dst]],
        [np.int16(0), np.float32(1.0), np.float32(1.0)],
    )
    iota = np.ascontiguousarray(np.tile(np.arange(128, dtype=np.float32), (128, 1)))
    iotac = np.arange(128, dtype=np.float32).reshape(128, 1)
    bembb = np.ascontiguousarray(np.tile(b_emb, (128, 1)))
    basnb = np.ascontiguousarray(np.tile(b_asn, (128, 1)))

    in_maps1 = []
    for c in range(NCORES):
        dloc, (gidx, dsg, dgd) = cores1[c]
        deg_own = deg[c * NPC : (c + 1) * NPC]
        in_maps1.append(
            {
                "x": x,
                "xown": np.ascontiguousarray(x[c * NPC : (c + 1) * NPC]),
                "idx1": _wrap16(gidx),
                "dloc1": _wrap128(dloc),
                "degs1": _wrap128(dsg),
                "degd1": _wrap128(dgd),
                "degown": np.ascontiguousarray(deg_own.reshape(BPC, 128).T),
                "iota": iota,
                "iotac": iotac,
                "wemb": W_emb,
                "wasn": W_asn,
                "bembb": bembb,
                "basnb": basnb,
            }
        )

    return cmax1, in_maps1, (src, dst)


def prep_pass2(S, src, dst, iota):
    codes = np.unique((src << 14) | dst)
    usrc = (codes >> 14).astype(np.int64)
    udst = (codes & (N_NODES - 1)).astype(np.int64)
    cmax2, cores2 = _group_edges(usrc, [udst.astype(np.int16)], [np.int16(0)])

    in_maps2 = []
    for c in range(NCORES):
        sloc, (gidx,) = cores2[c]
        in_maps2.append(
            {
                "sfull": S,
                "sown": np.ascontiguousarray(S[c * NPC : (c + 1) * NPC]),
                "idx2": _wrap16(gidx),
                "sloc2": _wrap128(sloc),
                "iota": iota,
            }
        )
    return cmax2, in_maps2


def kernel(x, edge_index, W_emb, b_emb, W_asn, b_asn):
    cmax1, in_maps1, (src, dst) = prep_pass1(x, edge_index, W_emb, b_emb, W_asn, b_asn)

    nc1 = _build_pass1(cmax1)
    res1 = run_bass_kernel_spmd(nc1, in_maps1, core_ids=list(range(NCORES)))
    S = np.concatenate([res1.results[c]["s_out"] for c in range(NCORES)], axis=0)
    x_pooled = np.sum([res1.results[c]["xp_part"] for c in range(NCORES)], axis=0)

    cmax2, in_maps2 = prep_pass2(S, src, dst, in_maps1[0]["iota"])
    nc2 = _build_pass2(cmax2)
    res2 = run_bass_kernel_spmd(nc2, in_maps2, core_ids=list(range(NCORES)))
    A_pooled = np.sum([res2.results[c]["ap_part"] for c in range(NCORES)], axis=0)

    global _LAST_SIM_NS
    _LAST_SIM_NS = {"nc1": nc1, "nc2": nc2}
    return x_pooled.astype(np.float32), A_pooled.astype(np.float32), S.astype(np.float32)


# revision 18
# speedup vs baseline: 1.0829x; 1.0829x over previous
"""DiffPool layer (GCN embed + GCN assign + softmax + S^T x / S^T A S) on 8 trn2 cores.

Sharding: nodes split into 8 contiguous ranges of 2048 (one per core).
Pass 1 (launch 1): edges sharded by dst owner; per 128-node block, messages
x[src]*w are gathered edge-major via dma_gather and segment-summed into the
block via weighted-one-hot matmuls accumulating in PSUM (out = gathered^T @ onehot
-> x_agg^T feature-major). Self-loops enter the same PSUM group as a weighted
diagonal matmul. Then logits/emb matmuls + softmax per block, and the
x_pooled = S^T x_emb partial accumulates across blocks in PSUM.
Pass 2 (launch 2): unique edges sharded by src owner; Z = A S built per block
from gathered S[dst] with unweighted one-hot matmuls (lhsT=onehot), then
A_pooled partial = S_own^T Z accumulates in PSUM. Host sums the 8 partials
and concatenates the S shards.
"""

import sys

sys.path.insert(0, "/opt/trn_rl_repo")

import numpy as np

import concourse.bacc as bacc
import concourse.mybir as mybir
from concourse.tile import TileContext
from concourse.bass_utils import run_bass_kernel_spmd

N_NODES = 16384
N_EDGES = 524288
F = 128  # in/hid/cluster channels all 128
NCORES = 8
NPC = N_NODES // NCORES  # nodes per core
BPC = NPC // 128  # 128-node blocks per core

AT = mybir.AluOpType
ACTF = mybir.ActivationFunctionType
F32 = mybir.dt.float32
I16 = mybir.dt.int16

_LAST_SIM_NS = {}


def _wrap16(a):
    """[n*16] int16 -> [128, n] gather-index layout (16-partition wrap, x8)."""
    t = a.reshape(-1, 16).T
    return np.ascontiguousarray(np.tile(t, (8, 1)))


def _wrap128(a):
    """[n*128] -> [128, n] edge-major (gather output position order)."""
    return np.ascontiguousarray(a.reshape(-1, 128).T)


def _group_edges(tgt, aux_arrays, n_aux_pad):
    """Group edges by target-node block, pad each (core, block) list to the
    shared per-block-position chunk count.

    tgt: [E] target node of each edge (the segment-sum key, already the
         sharding key). aux_arrays: list of per-edge arrays to carry along.
    n_aux_pad: pad value per aux array.
    Returns (cmax [BPC], per-core dict of arrays).
    """
    order = np.argsort(tgt, kind="stable")
    tgt_s = tgt[order]
    aux_s = [a[order] for a in aux_arrays]
    gb = tgt_s >> 7  # global block 0..127
    counts = np.bincount(gb, minlength=NCORES * BPC)
    starts = np.concatenate([[0], np.cumsum(counts)])
    cmax = np.zeros(BPC, dtype=np.int64)
    for c in range(NCORES):
        for b in range(BPC):
            cnt = counts[c * BPC + b]
            cmax[b] = max(cmax[b], -(-cnt // 128))
    cores = []
    for c in range(NCORES):
        loc_list = []  # local one-hot target (or -1 pad)
        aux_lists = [[] for _ in aux_arrays]
        for b in range(BPC):
            g = c * BPC + b
            s, e = starts[g], starts[g + 1]
            cnt = e - s
            pad = cmax[b] * 128 - cnt
            loc = (tgt_s[s:e] & 127).astype(np.float32)
            loc_list.append(np.concatenate([loc, np.full(pad, -1.0, np.float32)]))
            for ai, a in enumerate(aux_s):
                aux_lists[ai].append(
                    np.concatenate([a[s:e], np.full(pad, n_aux_pad[ai], a.dtype)])
                )
        cores.append(
            (np.concatenate(loc_list), [np.concatenate(al) for al in aux_lists])
        )
    return cmax, cores


def _build_pass1(cmax, zero_bias=False):
    CH1 = int(cmax.sum())
    nc = bacc.Bacc("TRN2", target_bir_lowering=False)
    xd = nc.dram_tensor("x", [N_NODES, F], F32, kind="ExternalInput")
    xod = nc.dram_tensor("xown", [NPC, F], F32, kind="ExternalInput")
    idxd = nc.dram_tensor("idx1", [128, CH1 * 8], I16, kind="ExternalInput")
    dlocd = nc.dram_tensor("dloc1", [128, CH1], F32, kind="ExternalInput")
    dsgd = nc.dram_tensor("degs1", [128, CH1], F32, kind="ExternalInput")
    dgdd = nc.dram_tensor("degd1", [128, CH1], F32, kind="ExternalInput")
    degod = nc.dram_tensor("degown", [128, BPC], F32, kind="ExternalInput")
    iotad = nc.dram_tensor("iota", [128, 128], F32, kind="ExternalInput")
    iotacd = nc.dram_tensor("iotac", [128, 1], F32, kind="ExternalInput")
    wembd = nc.dram_tensor("wemb", [F, F], F32, kind="ExternalInput")
    wasnd = nc.dram_tensor("wasn", [F, F], F32, kind="ExternalInput")
    bembd = nc.dram_tensor("bembb", [128, F], F32, kind="ExternalInput")
    basnd = nc.dram_tensor("basnb", [128, F], F32, kind="ExternalInput")
    sod = nc.dram_tensor("s_out", [NPC, F], F32, kind="ExternalOutput")
    xpd = nc.dram_tensor("xp_part", [128, 128], F32, kind="ExternalOutput")

    with TileContext(nc) as tc:
        with (
            tc.tile_pool(name="const", bufs=1) as constp,
            tc.tile_pool(name="meta", bufs=1) as metap,
            tc.tile_pool(name="gath", bufs=4) as gathp,
            tc.tile_pool(name="oh", bufs=4) as ohp,
            tc.tile_pool(name="work", bufs=4) as workp,
            tc.tile_pool(name="big", bufs=1) as bigp,
            tc.tile_pool(name="stat", bufs=4) as statp,
            tc.tile_pool(name="pagg", bufs=2, space="PSUM") as paggp,
            tc.tile_pool(name="pmm", bufs=2, space="PSUM") as pmmp,
            tc.tile_pool(name="pxp", bufs=1, space="PSUM") as pxpp,
        ):
            iota_t = constp.tile([128, 128], F32, tag="iota")
            nc.sync.dma_start(iota_t[:, :], iotad[:, :])
            iotac_t = constp.tile([128, 1], F32, tag="iotac")
            nc.sync.dma_start(iotac_t[:, :], iotacd[:, :])
            wemb_t = constp.tile([F, F], F32, tag="wemb")
            nc.sync.dma_start(wemb_t[:, :], wembd[:, :])
            wasn_t = constp.tile([F, F], F32, tag="wasn")
            nc.sync.dma_start(wasn_t[:, :], wasnd[:, :])
            bemb_t = constp.tile([128, F], F32, tag="bemb")
            nc.sync.dma_start(bemb_t[:, :], bembd[:, :])
            basn_t = constp.tile([128, F], F32, tag="basn")
            nc.sync.dma_start(basn_t[:, :], basnd[:, :])

            idx_t = metap.tile([128, CH1 * 8], I16, tag="idx")
            nc.sync.dma_start(idx_t[:, :], idxd[:, :])
            dloc_t = metap.tile([128, CH1], F32, tag="dloc")
            nc.sync.dma_start(dloc_t[:, :], dlocd[:, :])
            dsg_t = metap.tile([128, CH1], F32, tag="dsg")
            nc.sync.dma_start(dsg_t[:, :], dsgd[:, :])
            dgd_t = metap.tile([128, CH1], F32, tag="dgd")
            nc.sync.dma_start(dgd_t[:, :], dgdd[:, :])
            dego_t = metap.tile([128, BPC], F32, tag="dego")
            nc.sync.dma_start(dego_t[:, :], degod[:, :])

            # edge weights w = sqrt(1/(deg[src]*deg[dst]))
            dprod_t = metap.tile([128, CH1], F32, tag="dprod")
            nc.vector.tensor_tensor(dprod_t[:, :], dsg_t[:, :], dgd_t[:, :], AT.mult)
            drec_t = metap.tile([128, CH1], F32, tag="drec")
            nc.vector.reciprocal(drec_t[:, :], dprod_t[:, :])
            w1_t = metap.tile([128, CH1], F32, tag="w1")
            nc.scalar.activation(w1_t[:, :], drec_t[:, :], ACTF.Sqrt)
            # self-loop weight 1/deg
            invdeg_t = metap.tile([128, BPC], F32, tag="invdeg")
            nc.vector.reciprocal(invdeg_t[:, :], dego_t[:, :])

            xaggT = bigp.tile([128, NPC], F32, tag="xaggT")
            S_sb = bigp.tile([128, NPC], F32, tag="S_sb")
            xemb_sb = bigp.tile([128, NPC], F32, tag="xemb_sb")

            # gather calls capped at 1024 idxs (8 chunks) — HW limit; calls
            # are decoupled from blocks and cover the global chunk stream.
            CH1 = int(cmax.sum())
            CPG = 8  # chunks per gather call
            gtiles = {}

            def chunk_ap(ci):
                k = ci // CPG
                if k not in gtiles:
                    w = min(CPG, CH1 - k * CPG)
                    t = gathp.tile([128, w, F], F32, tag="g")
                    nc.gpsimd.dma_gather(
                        t[:, :, :],
                        xd[:, :],
                        idx_t[:, k * CPG * 8 : k * CPG * 8 + w * 8],
                        num_idxs=w * 128,
                        num_idxs_reg=w * 128,
                        elem_size=F,
                    )
                    gtiles[k] = t
                return gtiles[k][:, ci % CPG, :]

            # ---- aggregation: x_agg^T[f, v] blocks in PSUM ----
            off = 0
            for b in range(BPC):
                nch = int(cmax[b])
                bs = slice(b * 128, (b + 1) * 128)
                pa = paggp.tile([128, 128], F32, tag="pa")
                for ci in range(nch):
                    oh = ohp.tile([128, 128], F32, tag="oh")
                    col = slice(off + ci, off + ci + 1)
                    nc.vector.tensor_scalar(
                        oh[:, :], iota_t[:, :], dloc_t[:, col], w1_t[:, col],
                        AT.is_equal, AT.mult,
                    )
                    nc.tensor.matmul(
                        pa[:, :], chunk_ap(off + ci), oh[:, :],
                        start=(ci == 0), stop=False, skip_group_check=True,
                    )
                # self loop: lhsT = x_own block [v(e), f], rhs = diag(1/deg)
                dmat = ohp.tile([128, 128], F32, tag="oh")
                nc.vector.tensor_scalar(
                    dmat[:, :], iota_t[:, :], iotac_t[:, :], invdeg_t[:, b : b + 1],
                    AT.is_equal, AT.mult,
                )
                xob = workp.tile([128, F], F32, tag="xob")
                nc.sync.dma_start(xob[:, :], xod[b * 128 : (b + 1) * 128, :])
                nc.tensor.matmul(
                    pa[:, :], xob[:, :], dmat[:, :],
                    start=False, stop=True, skip_group_check=True,
                )
                nc.vector.tensor_copy(xaggT[:, bs], pa[:, :])
                off += nch

            # ---- per-block: logits -> softmax -> S ; emb -> relu ; xp accum ----
            pxp = pxpp.tile([128, 128], F32, tag="pxp")
            for b in range(BPC):
                bs = slice(b * 128, (b + 1) * 128)
                xat = xaggT[:, bs]
                pl = pmmp.tile([128, 128], F32, tag="pl")
                nc.tensor.matmul(pl[:, :], xat, wasn_t[:, :], start=True, stop=True,
                                 skip_group_check=True)
                lg = workp.tile([128, 128], F32, tag="lg")
                nc.vector.tensor_tensor(lg[:, :], pl[:, :], basn_t[:, :], AT.add)
                negm = statp.tile([128, 1], F32, tag="negm")
                nc.vector.tensor_reduce(
                    negm[:, :], lg[:, :], axis=mybir.AxisListType.X, op=AT.max,
                    negate=True,
                )
                ex = workp.tile([128, 128], F32, tag="ex")
                ssum = statp.tile([128, 1], F32, tag="ssum")
                nc.scalar.activation(
                    ex[:, :], lg[:, :], ACTF.Exp, bias=negm[:, :], accum_out=ssum[:, :]
                )
                rs = statp.tile([128, 1], F32, tag="rs")
                nc.vector.reciprocal(rs[:, :], ssum[:, :])
                nc.vector.tensor_scalar_mul(S_sb[:, bs], ex[:, :], rs[:, :])
                nc.sync.dma_start(sod[b * 128 : (b + 1) * 128, :], S_sb[:, bs])

                pe_ = pmmp.tile([128, 128], F32, tag="pe")
                nc.tensor.matmul(pe_[:, :], xat, wemb_t[:, :], start=True, stop=True,
                                 skip_group_check=True)
                t2 = workp.tile([128, 128], F32, tag="t2")
                nc.vector.tensor_tensor(t2[:, :], pe_[:, :], bemb_t[:, :], AT.add)
                nc.vector.tensor_scalar_max(xemb_sb[:, bs], t2[:, :], 0.0)
                nc.tensor.matmul(
                    pxp[:, :], S_sb[:, bs], xemb_sb[:, bs],
                    start=(b == 0), stop=(b == BPC - 1), skip_group_check=True,
                )
            xps = workp.tile([128, 128], F32, tag="xps")
            nc.vector.tensor_copy(xps[:, :], pxp[:, :])
            nc.sync.dma_start(xpd[:, :], xps[:, :])

    nc.compile()
    return nc


def _build_pass2(cmax2):
    BF16 = mybir.dt.bfloat16
    CH2 = int(cmax2.sum())
    nc = bacc.Bacc("TRN2", target_bir_lowering=False)
    # gathered S and the 0/1 one-hots are bf16 (one-hots exact; S rounding
    # washes out to ~1e-4 on A_pooled) — halves pass-2 gather traffic.
    sfd = nc.dram_tensor("sfull", [N_NODES, F], BF16, kind="ExternalInput")
    sod = nc.dram_tensor("sown", [NPC, F], F32, kind="ExternalInput")
    idxd = nc.dram_tensor("idx2", [128, CH2 * 8], I16, kind="ExternalInput")
    slocd = nc.dram_tensor("sloc2", [128, CH2], F32, kind="ExternalInput")
    iotad = nc.dram_tensor("iota", [128, 128], F32, kind="ExternalInput")
    apd = nc.dram_tensor("ap_part", [128, 128], F32, kind="ExternalOutput")

    with TileContext(nc) as tc:
        with (
            tc.tile_pool(name="const", bufs=1) as constp,
            tc.tile_pool(name="meta", bufs=1) as metap,
            tc.tile_pool(name="gath", bufs=4) as gathp,
            tc.tile_pool(name="oh", bufs=4) as ohp,
            tc.tile_pool(name="work", bufs=4) as workp,
            tc.tile_pool(name="pz", bufs=2, space="PSUM") as pzp,
            tc.tile_pool(name="pap", bufs=1, space="PSUM") as papp,
        ):
            iota_t = constp.tile([128, 128], F32, tag="iota")
            nc.sync.dma_start(iota_t[:, :], iotad[:, :])
            idx_t = metap.tile([128, CH2 * 8], I16, tag="idx")
            nc.sync.dma_start(idx_t[:, :], idxd[:, :])
            sloc_t = metap.tile([128, CH2], F32, tag="sloc")
            nc.sync.dma_start(sloc_t[:, :], slocd[:, :])

            CH2 = int(cmax2.sum())
            CPG = 8
            gtiles = {}

            def chunk_ap(ci):
                k = ci // CPG
                if k not in gtiles:
                    w = min(CPG, CH2 - k * CPG)
                    t = gathp.tile([128, w, F], BF16, tag="g")
                    nc.gpsimd.dma_gather(
                        t[:, :, :],
                        sfd[:, :],
                        idx_t[:, k * CPG * 8 : k * CPG * 8 + w * 8],
                        num_idxs=w * 128,
                        num_idxs_reg=w * 128,
                        elem_size=F,
                    )
                    gtiles[k] = t
                return gtiles[k][:, ci % CPG, :]

            pap = papp.tile([128, 128], F32, tag="pap")
            off = 0
            for b in range(BPC):
                nch = int(cmax2[b])
                pz = pzp.tile([128, 128], F32, tag="pz")
                for ci in range(nch):
                    oh = ohp.tile([128, 128], BF16, tag="oh")
                    col = slice(off + ci, off + ci + 1)
                    nc.vector.tensor_scalar(
                        oh[:, :], iota_t[:, :], sloc_t[:, col], None, AT.is_equal
                    )
                    # lhsT = onehot [e, v], rhs = gathered S[dst] [e, l] -> Z[v, l]
                    nc.tensor.matmul(
                        pz[:, :], oh[:, :], chunk_ap(off + ci),
                        start=(ci == 0), stop=(ci == nch - 1), skip_group_check=True,
                    )
                zb = workp.tile([128, 128], F32, tag="zb")
                nc.vector.tensor_copy(zb[:, :], pz[:, :])
                sob = workp.tile([128, F], F32, tag="sob")
                nc.sync.dma_start(sob[:, :], sod[b * 128 : (b + 1) * 128, :])
                nc.tensor.matmul(
                    pap[:, :], sob[:, :], zb[:, :],
                    start=(b == 0), stop=(b == BPC - 1), skip_group_check=True,
                )
                off += nch
            aps = workp.tile([128, 128], F32, tag="aps")
            nc.vector.tensor_copy(aps[:, :], pap[:, :])
            nc.sync.dma_start(apd[:, :], aps[:, :])

    nc.compile()
    return nc


def prep_pass1(x, edge_index, W_emb, b_emb, W_asn, b_asn):
    x = np.ascontiguousarray(np.asarray(x, dtype=np.float32))
    ei = np.asarray(edge_index)
    src = ei[0].astype(np.int64)
    dst = ei[1].astype(np.int64)
    W_emb = np.ascontiguousarray(np.asarray(W_emb, np.float32))
    W_asn = np.ascontiguousarray(np.asarray(W_asn, np.float32))
    b_emb = np.asarray(b_emb, np.float32)
    b_asn = np.asarray(b_asn, np.float32)

    deg = (np.bincount(dst, minlength=N_NODES) + 1).astype(np.float32)

    # ---------- pass 1 host prep: edges grouped by dst ----------
    cmax1, cores1 = _group_edges(
        dst,
        [src.astype(np.int16), deg[src], deg[dst]],
        [np.int16(0), np.float32(1.0), np.float32(1.0)],
    )
    iota = np.ascontiguousarray(np.tile(np.arange(128, dtype=np.float32), (128, 1)))
    iotac = np.arange(128, dtype=np.float32).reshape(128, 1)
    bembb = np.ascontiguousarray(np.tile(b_emb, (128, 1)))
    basnb = np.ascontiguousarray(np.tile(b_asn, (128, 1)))

    in_maps1 = []
    for c in range(NCORES):
        dloc, (gidx, dsg, dgd) = cores1[c]
        deg_own = deg[c * NPC : (c + 1) * NPC]
        in_maps1.append(
            {
                "x": x,
                "xown": np.ascontiguousarray(x[c * NPC : (c + 1) * NPC]),
                "idx1": _wrap16(gidx),
                "dloc1": _wrap128(dloc),
                "degs1": _wrap128(dsg),
                "degd1": _wrap128(dgd),
                "degown": np.ascontiguousarray(deg_own.reshape(BPC, 128).T),
                "iota": iota,
                "iotac": iotac,
                "wemb": W_emb,
                "wasn": W_asn,
                "bembb": bembb,
                "basnb": basnb,
            }
        )

    return cmax1, in_maps1, (src, dst)


def prep_pass2(S, src, dst, iota):
    import ml_dtypes

    codes = np.unique((src << 14) | dst)
    usrc = (codes >> 14).astype(np.int64)
    udst = (codes & (N_NODES - 1)).astype(np.int64)
    cmax2, cores2 = _group_edges(usrc, [udst.astype(np.int16)], [np.int16(0)])

    S_bf16 = np.ascontiguousarray(S.astype(ml_dtypes.bfloat16))
    in_maps2 = []
    for c in range(NCORES):
        sloc, (gidx,) = cores2[c]
        in_maps2.append(
            {
                "sfull": S_bf16,
                "sown": np.ascontiguousarray(S[c * NPC : (c + 1) * NPC]),
                "idx2": _wrap16(gidx),
                "sloc2": _wrap128(sloc),
                "iota": iota,
            }
        )
    return cmax2, in_maps2


def kernel(x, edge_index, W_emb, b_emb, W_asn, b_asn):
    cmax1, in_maps1, (src, dst) = prep_pass1(x, edge_index, W_emb, b_emb, W_asn, b_asn)

    nc1 = _build_pass1(cmax1)
    res1 = run_bass_kernel_spmd(nc1, in_maps1, core_ids=list(range(NCORES)))
    S = np.concatenate([res1.results[c]["s_out"] for c in range(NCORES)], axis=0)
    x_pooled = np.sum([res1.results[c]["xp_part"] for c in range(NCORES)], axis=0)

    cmax2, in_maps2 = prep_pass2(S, src, dst, in_maps1[0]["iota"])
    nc2 = _build_pass2(cmax2)
    res2 = run_bass_kernel_spmd(nc2, in_maps2, core_ids=list(range(NCORES)))
    A_pooled = np.sum([res2.results[c]["ap_part"] for c in range(NCORES)], axis=0)

    global _LAST_SIM_NS
    _LAST_SIM_NS = {"nc1": nc1, "nc2": nc2}
    return x_pooled.astype(np.float32), A_pooled.astype(np.float32), S.astype(np.float32)


# revision 19
# speedup vs baseline: 1.1829x; 1.0924x over previous
"""DiffPool layer (GCN embed + GCN assign + softmax + S^T x / S^T A S) on 8 trn2 cores.

Sharding: nodes split into 8 contiguous ranges of 2048 (one per core).
Pass 1 (launch 1): edges sharded by dst owner; per 128-node block, messages
x[src]*w are gathered edge-major via dma_gather and segment-summed into the
block via weighted-one-hot matmuls accumulating in PSUM (out = gathered^T @ onehot
-> x_agg^T feature-major). Self-loops enter the same PSUM group as a weighted
diagonal matmul. Then logits/emb matmuls + softmax per block, and the
x_pooled = S^T x_emb partial accumulates across blocks in PSUM.
Pass 2 (launch 2): unique edges sharded by src owner; Z = A S built per block
from gathered S[dst] with unweighted one-hot matmuls (lhsT=onehot), then
A_pooled partial = S_own^T Z accumulates in PSUM. Host sums the 8 partials
and concatenates the S shards.
"""

import sys

sys.path.insert(0, "/opt/trn_rl_repo")

import numpy as np

import concourse.bacc as bacc
import concourse.mybir as mybir
from concourse.tile import TileContext
from concourse.bass_utils import run_bass_kernel_spmd

N_NODES = 16384
N_EDGES = 524288
F = 128  # in/hid/cluster channels all 128
NCORES = 8
NPC = N_NODES // NCORES  # nodes per core
BPC = NPC // 128  # 128-node blocks per core

AT = mybir.AluOpType
ACTF = mybir.ActivationFunctionType
F32 = mybir.dt.float32
I16 = mybir.dt.int16

_LAST_SIM_NS = {}


def _wrap16(a):
    """[n*16] int16 -> [128, n] gather-index layout (16-partition wrap, x8)."""
    t = a.reshape(-1, 16).T
    return np.ascontiguousarray(np.tile(t, (8, 1)))


def _wrap128(a):
    """[n*128] -> [128, n] edge-major (gather output position order)."""
    return np.ascontiguousarray(a.reshape(-1, 128).T)


def _group_edges(tgt, aux_arrays, n_aux_pad):
    """Group edges by target-node block, pad each (core, block) list to the
    shared per-block-position chunk count.

    tgt: [E] target node of each edge (the segment-sum key, already the
         sharding key). aux_arrays: list of per-edge arrays to carry along.
    n_aux_pad: pad value per aux array.
    Returns (cmax [BPC], per-core dict of arrays).
    """
    order = np.argsort(tgt, kind="stable")
    tgt_s = tgt[order]
    aux_s = [a[order] for a in aux_arrays]
    gb = tgt_s >> 7  # global block 0..127
    counts = np.bincount(gb, minlength=NCORES * BPC)
    starts = np.concatenate([[0], np.cumsum(counts)])
    cmax = np.zeros(BPC, dtype=np.int64)
    for c in range(NCORES):
        for b in range(BPC):
            cnt = counts[c * BPC + b]
            cmax[b] = max(cmax[b], -(-cnt // 128))
    cores = []
    for c in range(NCORES):
        loc_list = []  # local one-hot target (or -1 pad)
        aux_lists = [[] for _ in aux_arrays]
        for b in range(BPC):
            g = c * BPC + b
            s, e = starts[g], starts[g + 1]
            cnt = e - s
            pad = cmax[b] * 128 - cnt
            loc = (tgt_s[s:e] & 127).astype(np.float32)
            loc_list.append(np.concatenate([loc, np.full(pad, -1.0, np.float32)]))
            for ai, a in enumerate(aux_s):
                aux_lists[ai].append(
                    np.concatenate([a[s:e], np.full(pad, n_aux_pad[ai], a.dtype)])
                )
        cores.append(
            (np.concatenate(loc_list), [np.concatenate(al) for al in aux_lists])
        )
    return cmax, cores


def _build_pass1(cmax, zero_bias=False):
    BF16 = mybir.dt.bfloat16
    CH1 = int(cmax.sum())
    nc = bacc.Bacc("TRN2", target_bir_lowering=False)
    # x is only the gather source here; bf16 halves the 36MB/core gather
    # leg. One-hot weights/diag also bf16 (matmul operand-dtype pairing).
    xd = nc.dram_tensor("x", [N_NODES, F], BF16, kind="ExternalInput")
    xod = nc.dram_tensor("xown", [NPC, F], BF16, kind="ExternalInput")
    idxd = nc.dram_tensor("idx1", [128, CH1 * 8], I16, kind="ExternalInput")
    dlocd = nc.dram_tensor("dloc1", [128, CH1], F32, kind="ExternalInput")
    dsgd = nc.dram_tensor("degs1", [128, CH1], F32, kind="ExternalInput")
    dgdd = nc.dram_tensor("degd1", [128, CH1], F32, kind="ExternalInput")
    degod = nc.dram_tensor("degown", [128, BPC], F32, kind="ExternalInput")
    iotad = nc.dram_tensor("iota", [128, 128], F32, kind="ExternalInput")
    iotacd = nc.dram_tensor("iotac", [128, 1], F32, kind="ExternalInput")
    wembd = nc.dram_tensor("wemb", [F, F], F32, kind="ExternalInput")
    wasnd = nc.dram_tensor("wasn", [F, F], F32, kind="ExternalInput")
    bembd = nc.dram_tensor("bembb", [128, F], F32, kind="ExternalInput")
    basnd = nc.dram_tensor("basnb", [128, F], F32, kind="ExternalInput")
    sod = nc.dram_tensor("s_out", [NPC, F], F32, kind="ExternalOutput")
    xpd = nc.dram_tensor("xp_part", [128, 128], F32, kind="ExternalOutput")

    with TileContext(nc) as tc:
        with (
            tc.tile_pool(name="const", bufs=1) as constp,
            tc.tile_pool(name="meta", bufs=1) as metap,
            tc.tile_pool(name="gath", bufs=4) as gathp,
            tc.tile_pool(name="oh", bufs=4) as ohp,
            tc.tile_pool(name="work", bufs=4) as workp,
            tc.tile_pool(name="big", bufs=1) as bigp,
            tc.tile_pool(name="stat", bufs=4) as statp,
            tc.tile_pool(name="pagg", bufs=2, space="PSUM") as paggp,
            tc.tile_pool(name="pmm", bufs=2, space="PSUM") as pmmp,
            tc.tile_pool(name="pxp", bufs=1, space="PSUM") as pxpp,
        ):
            iota_t = constp.tile([128, 128], F32, tag="iota")
            nc.sync.dma_start(iota_t[:, :], iotad[:, :])
            iotac_t = constp.tile([128, 1], F32, tag="iotac")
            nc.sync.dma_start(iotac_t[:, :], iotacd[:, :])
            wemb_t = constp.tile([F, F], F32, tag="wemb")
            nc.sync.dma_start(wemb_t[:, :], wembd[:, :])
            wasn_t = constp.tile([F, F], F32, tag="wasn")
            nc.sync.dma_start(wasn_t[:, :], wasnd[:, :])
            bemb_t = constp.tile([128, F], F32, tag="bemb")
            nc.sync.dma_start(bemb_t[:, :], bembd[:, :])
            basn_t = constp.tile([128, F], F32, tag="basn")
            nc.sync.dma_start(basn_t[:, :], basnd[:, :])

            idx_t = metap.tile([128, CH1 * 8], I16, tag="idx")
            nc.sync.dma_start(idx_t[:, :], idxd[:, :])
            dloc_t = metap.tile([128, CH1], F32, tag="dloc")
            nc.sync.dma_start(dloc_t[:, :], dlocd[:, :])
            dsg_t = metap.tile([128, CH1], F32, tag="dsg")
            nc.sync.dma_start(dsg_t[:, :], dsgd[:, :])
            dgd_t = metap.tile([128, CH1], F32, tag="dgd")
            nc.sync.dma_start(dgd_t[:, :], dgdd[:, :])
            dego_t = metap.tile([128, BPC], F32, tag="dego")
            nc.sync.dma_start(dego_t[:, :], degod[:, :])

            # edge weights w = sqrt(1/(deg[src]*deg[dst]))
            dprod_t = metap.tile([128, CH1], F32, tag="dprod")
            nc.vector.tensor_tensor(dprod_t[:, :], dsg_t[:, :], dgd_t[:, :], AT.mult)
            drec_t = metap.tile([128, CH1], F32, tag="drec")
            nc.vector.reciprocal(drec_t[:, :], dprod_t[:, :])
            w1_t = metap.tile([128, CH1], F32, tag="w1")
            nc.scalar.activation(w1_t[:, :], drec_t[:, :], ACTF.Sqrt)
            # self-loop weight 1/deg
            invdeg_t = metap.tile([128, BPC], F32, tag="invdeg")
            nc.vector.reciprocal(invdeg_t[:, :], dego_t[:, :])

            xaggT = bigp.tile([128, NPC], F32, tag="xaggT")
            S_sb = bigp.tile([128, NPC], F32, tag="S_sb")
            xemb_sb = bigp.tile([128, NPC], F32, tag="xemb_sb")

            # gather calls capped at 1024 idxs (8 chunks) — HW limit; calls
            # are decoupled from blocks and cover the global chunk stream.
            CH1 = int(cmax.sum())
            CPG = 8  # chunks per gather call
            gtiles = {}

            def chunk_ap(ci):
                k = ci // CPG
                if k not in gtiles:
                    w = min(CPG, CH1 - k * CPG)
                    t = gathp.tile([128, w, F], BF16, tag="g")
                    nc.gpsimd.dma_gather(
                        t[:, :, :],
                        xd[:, :],
                        idx_t[:, k * CPG * 8 : k * CPG * 8 + w * 8],
                        num_idxs=w * 128,
                        num_idxs_reg=w * 128,
                        elem_size=F,
                    )
                    gtiles[k] = t
                return gtiles[k][:, ci % CPG, :]

            # ---- aggregation: x_agg^T[f, v] blocks in PSUM ----
            off = 0
            for b in range(BPC):
                nch = int(cmax[b])
                bs = slice(b * 128, (b + 1) * 128)
                pa = paggp.tile([128, 128], F32, tag="pa")
                for ci in range(nch):
                    oh = ohp.tile([128, 128], BF16, tag="oh")
                    col = slice(off + ci, off + ci + 1)
                    nc.vector.tensor_scalar(
                        oh[:, :], iota_t[:, :], dloc_t[:, col], w1_t[:, col],
                        AT.is_equal, AT.mult,
                    )
                    nc.tensor.matmul(
                        pa[:, :], chunk_ap(off + ci), oh[:, :],
                        start=(ci == 0), stop=False, skip_group_check=True,
                    )
                # self loop: lhsT = x_own block [v(e), f], rhs = diag(1/deg)
                dmat = ohp.tile([128, 128], BF16, tag="oh")
                nc.vector.tensor_scalar(
                    dmat[:, :], iota_t[:, :], iotac_t[:, :], invdeg_t[:, b : b + 1],
                    AT.is_equal, AT.mult,
                )
                xob = workp.tile([128, F], BF16, tag="xob")
                nc.sync.dma_start(xob[:, :], xod[b * 128 : (b + 1) * 128, :])
                nc.tensor.matmul(
                    pa[:, :], xob[:, :], dmat[:, :],
                    start=False, stop=True, skip_group_check=True,
                )
                nc.vector.tensor_copy(xaggT[:, bs], pa[:, :])
                off += nch

            # ---- per-block: logits -> softmax -> S ; emb -> relu ; xp accum ----
            pxp = pxpp.tile([128, 128], F32, tag="pxp")
            for b in range(BPC):
                bs = slice(b * 128, (b + 1) * 128)
                xat = xaggT[:, bs]
                pl = pmmp.tile([128, 128], F32, tag="pl")
                nc.tensor.matmul(pl[:, :], xat, wasn_t[:, :], start=True, stop=True,
                                 skip_group_check=True)
                lg = workp.tile([128, 128], F32, tag="lg")
                nc.vector.tensor_tensor(lg[:, :], pl[:, :], basn_t[:, :], AT.add)
                negm = statp.tile([128, 1], F32, tag="negm")
                nc.vector.tensor_reduce(
                    negm[:, :], lg[:, :], axis=mybir.AxisListType.X, op=AT.max,
                    negate=True,
                )
                ex = workp.tile([128, 128], F32, tag="ex")
                ssum = statp.tile([128, 1], F32, tag="ssum")
                nc.scalar.activation(
                    ex[:, :], lg[:, :], ACTF.Exp, bias=negm[:, :], accum_out=ssum[:, :]
                )
                rs = statp.tile([128, 1], F32, tag="rs")
                nc.vector.reciprocal(rs[:, :], ssum[:, :])
                nc.vector.tensor_scalar_mul(S_sb[:, bs], ex[:, :], rs[:, :])
                nc.sync.dma_start(sod[b * 128 : (b + 1) * 128, :], S_sb[:, bs])

                pe_ = pmmp.tile([128, 128], F32, tag="pe")
                nc.tensor.matmul(pe_[:, :], xat, wemb_t[:, :], start=True, stop=True,
                                 skip_group_check=True)
                t2 = workp.tile([128, 128], F32, tag="t2")
                nc.vector.tensor_tensor(t2[:, :], pe_[:, :], bemb_t[:, :], AT.add)
                nc.vector.tensor_scalar_max(xemb_sb[:, bs], t2[:, :], 0.0)
                nc.tensor.matmul(
                    pxp[:, :], S_sb[:, bs], xemb_sb[:, bs],
                    start=(b == 0), stop=(b == BPC - 1), skip_group_check=True,
                )
            xps = workp.tile([128, 128], F32, tag="xps")
            nc.vector.tensor_copy(xps[:, :], pxp[:, :])
            nc.sync.dma_start(xpd[:, :], xps[:, :])

    nc.compile()
    return nc


def _build_pass2(cmax2):
    BF16 = mybir.dt.bfloat16
    CH2 = int(cmax2.sum())
    nc = bacc.Bacc("TRN2", target_bir_lowering=False)
    # gathered S and the 0/1 one-hots are bf16 (one-hots exact; S rounding
    # washes out to ~1e-4 on A_pooled) — halves pass-2 gather traffic.
    sfd = nc.dram_tensor("sfull", [N_NODES, F], BF16, kind="ExternalInput")
    sod = nc.dram_tensor("sown", [NPC, F], F32, kind="ExternalInput")
    idxd = nc.dram_tensor("idx2", [128, CH2 * 8], I16, kind="ExternalInput")
    slocd = nc.dram_tensor("sloc2", [128, CH2], F32, kind="ExternalInput")
    iotad = nc.dram_tensor("iota", [128, 128], F32, kind="ExternalInput")
    apd = nc.dram_tensor("ap_part", [128, 128], F32, kind="ExternalOutput")

    with TileContext(nc) as tc:
        with (
            tc.tile_pool(name="const", bufs=1) as constp,
            tc.tile_pool(name="meta", bufs=1) as metap,
            tc.tile_pool(name="gath", bufs=4) as gathp,
            tc.tile_pool(name="oh", bufs=4) as ohp,
            tc.tile_pool(name="work", bufs=4) as workp,
            tc.tile_pool(name="pz", bufs=2, space="PSUM") as pzp,
            tc.tile_pool(name="pap", bufs=1, space="PSUM") as papp,
        ):
            iota_t = constp.tile([128, 128], F32, tag="iota")
            nc.sync.dma_start(iota_t[:, :], iotad[:, :])
            idx_t = metap.tile([128, CH2 * 8], I16, tag="idx")
            nc.sync.dma_start(idx_t[:, :], idxd[:, :])
            sloc_t = metap.tile([128, CH2], F32, tag="sloc")
            nc.sync.dma_start(sloc_t[:, :], slocd[:, :])

            CH2 = int(cmax2.sum())
            CPG = 8
            gtiles = {}

            def chunk_ap(ci):
                k = ci // CPG
                if k not in gtiles:
                    w = min(CPG, CH2 - k * CPG)
                    t = gathp.tile([128, w, F], BF16, tag="g")
                    nc.gpsimd.dma_gather(
                        t[:, :, :],
                        sfd[:, :],
                        idx_t[:, k * CPG * 8 : k * CPG * 8 + w * 8],
                        num_idxs=w * 128,
                        num_idxs_reg=w * 128,
                        elem_size=F,
                    )
                    gtiles[k] = t
                return gtiles[k][:, ci % CPG, :]

            pap = papp.tile([128, 128], F32, tag="pap")
            off = 0
            for b in range(BPC):
                nch = int(cmax2[b])
                pz = pzp.tile([128, 128], F32, tag="pz")
                for ci in range(nch):
                    oh = ohp.tile([128, 128], BF16, tag="oh")
                    col = slice(off + ci, off + ci + 1)
                    nc.vector.tensor_scalar(
                        oh[:, :], iota_t[:, :], sloc_t[:, col], None, AT.is_equal
                    )
                    # lhsT = onehot [e, v], rhs = gathered S[dst] [e, l] -> Z[v, l]
                    nc.tensor.matmul(
                        pz[:, :], oh[:, :], chunk_ap(off + ci),
                        start=(ci == 0), stop=(ci == nch - 1), skip_group_check=True,
                    )
                zb = workp.tile([128, 128], F32, tag="zb")
                nc.vector.tensor_copy(zb[:, :], pz[:, :])
                sob = workp.tile([128, F], F32, tag="sob")
                nc.sync.dma_start(sob[:, :], sod[b * 128 : (b + 1) * 128, :])
                nc.tensor.matmul(
                    pap[:, :], sob[:, :], zb[:, :],
                    start=(b == 0), stop=(b == BPC - 1), skip_group_check=True,
                )
                off += nch
            aps = workp.tile([128, 128], F32, tag="aps")
            nc.vector.tensor_copy(aps[:, :], pap[:, :])
            nc.sync.dma_start(apd[:, :], aps[:, :])

    nc.compile()
    return nc


def prep_pass1(x, edge_index, W_emb, b_emb, W_asn, b_asn):
    import ml_dtypes

    x = np.ascontiguousarray(np.asarray(x, dtype=np.float32))
    ei = np.asarray(edge_index)
    src = ei[0].astype(np.int64)
    dst = ei[1].astype(np.int64)
    W_emb = np.ascontiguousarray(np.asarray(W_emb, np.float32))
    W_asn = np.ascontiguousarray(np.asarray(W_asn, np.float32))
    b_emb = np.asarray(b_emb, np.float32)
    b_asn = np.asarray(b_asn, np.float32)

    deg = (np.bincount(dst, minlength=N_NODES) + 1).astype(np.float32)

    # ---------- pass 1 host prep: edges grouped by dst ----------
    cmax1, cores1 = _group_edges(
        dst,
        [src.astype(np.int16), deg[src], deg[dst]],
        [np.int16(0), np.float32(1.0), np.float32(1.0)],
    )
    iota = np.ascontiguousarray(np.tile(np.arange(128, dtype=np.float32), (128, 1)))
    iotac = np.arange(128, dtype=np.float32).reshape(128, 1)
    bembb = np.ascontiguousarray(np.tile(b_emb, (128, 1)))
    basnb = np.ascontiguousarray(np.tile(b_asn, (128, 1)))

    in_maps1 = []
    for c in range(NCORES):
        dloc, (gidx, dsg, dgd) = cores1[c]
        deg_own = deg[c * NPC : (c + 1) * NPC]
        x_bf16 = np.ascontiguousarray(x.astype(ml_dtypes.bfloat16))
        in_maps1.append(
            {
                "x": x_bf16,
                "xown": np.ascontiguousarray(x_bf16[c * NPC : (c + 1) * NPC]),
                "idx1": _wrap16(gidx),
                "dloc1": _wrap128(dloc),
                "degs1": _wrap128(dsg),
                "degd1": _wrap128(dgd),
                "degown": np.ascontiguousarray(deg_own.reshape(BPC, 128).T),
                "iota": iota,
                "iotac": iotac,
                "wemb": W_emb,
                "wasn": W_asn,
                "bembb": bembb,
                "basnb": basnb,
            }
        )

    return cmax1, in_maps1, (src, dst)


def prep_pass2(S, src, dst, iota):
    import ml_dtypes

    codes = np.unique((src << 14) | dst)
    usrc = (codes >> 14).astype(np.int64)
    udst = (codes & (N_NODES - 1)).astype(np.int64)
    cmax2, cores2 = _group_edges(usrc, [udst.astype(np.int16)], [np.int16(0)])

    S_bf16 = np.ascontiguousarray(S.astype(ml_dtypes.bfloat16))
    in_maps2 = []
    for c in range(NCORES):
        sloc, (gidx,) = cores2[c]
        in_maps2.append(
            {
                "sfull": S_bf16,
                "sown": np.ascontiguousarray(S[c * NPC : (c + 1) * NPC]),
                "idx2": _wrap16(gidx),
                "sloc2": _wrap128(sloc),
                "iota": iota,
            }
        )
    return cmax2, in_maps2


def kernel(x, edge_index, W_emb, b_emb, W_asn, b_asn):
    cmax1, in_maps1, (src, dst) = prep_pass1(x, edge_index, W_emb, b_emb, W_asn, b_asn)

    nc1 = _build_pass1(cmax1)
    res1 = run_bass_kernel_spmd(nc1, in_maps1, core_ids=list(range(NCORES)))
    S = np.concatenate([res1.results[c]["s_out"] for c in range(NCORES)], axis=0)
    x_pooled = np.sum([res1.results[c]["xp_part"] for c in range(NCORES)], axis=0)

    cmax2, in_maps2 = prep_pass2(S, src, dst, in_maps1[0]["iota"])
    nc2 = _build_pass2(cmax2)
    res2 = run_bass_kernel_spmd(nc2, in_maps2, core_ids=list(range(NCORES)))
    A_pooled = np.sum([res2.results[c]["ap_part"] for c in range(NCORES)], axis=0)

    global _LAST_SIM_NS
    _LAST_SIM_NS = {"nc1": nc1, "nc2": nc2}
    return x_pooled.astype(np.float32), A_pooled.astype(np.float32), S.astype(np.float32)
